# revision 7
# baseline (speedup 1.0000x reference)
"""Trainium2 Bass kernel for a pre-LN attention block.

Reference computation (B=2, L=2048, D=1024, H=16, hd=64):
    h = LayerNorm(x) * gamma + beta
    q, k, v = h @ W{q,k,v}.T + b{q,k,v}      (split into 16 heads of 64)
    o = softmax(q k^T / sqrt(hd)) v
    out = x + (o @ Wo.T + bo)

Sharding over 8 cores: core c handles batch b = c // 4 and head group
g = c % 4 (4 heads, 256 hidden dims).  Each core computes a partial
output  Ypart = attn_heads_g(LN(x[b])) @ Wo[:, g]T ; the host sums the
four partials per batch and adds the residual and bo in fp32.

Device-side layout is fully "feature-on-partitions" (transposed):
  - xT [D, L] bf16 arrives pre-transposed from host.
  - LN row stats (mean, rsqrt(var)) are computed with ones-matmuls on
    the tensor engine; normalization is hTs = xT * a_row (a = rsqrt),
    while the mean term (and LN beta / projection biases, gamma is
    folded into the weights host-side) enters each projection as a K=2
    correction matmul accumulated into the same PSUM group.
  - qT/kT [256, L]; v in natural layout [L, 4, 65] with a ones column.
  - Per head: ST = k q^T -> PSUM; exp(ST/8) is the PSUM->SBUF eviction
    on the scalar engine; OT' = [V|1]^T @ exp(ST) gives both the head
    output and the softmax denominator (row 64); normalization happens
    during OT' eviction via a broadcast reciprocal row.
"""

import numpy as np
import ml_dtypes

BF16 = ml_dtypes.bfloat16

B, L, D = 2, 2048, 1024
H, HD = 16, 64
HG = 4                 # head groups (cores per batch)
GH = H // HG           # heads per group = 4
GD = GH * HD           # hidden dims per group = 256
N_CORES = 8
PART = 128
NB = 512               # matmul moving free dim / PSUM bank width (fp32)
DC = D // PART         # 8 chunks of the contraction dim
LT = L // PART         # 16 L-tiles
EPS = 1e-5


def _build_program(n_iter: int = 1):
    """Build + compile the SPMD single-core program. n_iter > 1 wraps the
    whole computation in a hardware loop (for slope-based timing)."""
    import concourse.bass as bass
    import concourse.bacc as bacc
    import concourse.tile as tile
    import concourse.mybir as mybir
    from concourse.engine_type import EngineType

    f32 = mybir.dt.float32
    bf16 = mybir.dt.bfloat16
    AF = mybir.ActivationFunctionType

    nc = bacc.Bacc("TRN2", target_bir_lowering=False, debug=False)

    xT_d = nc.dram_tensor("xT", [D, L], bf16, kind="ExternalInput")
    wqT_d = nc.dram_tensor("wqT", [D, GD], bf16, kind="ExternalInput")
    wkT_d = nc.dram_tensor("wkT", [D, GD], bf16, kind="ExternalInput")
    wvT_d = nc.dram_tensor("wvT", [D, GD], bf16, kind="ExternalInput")
    woT_d = nc.dram_tensor("woT", [GD, D], bf16, kind="ExternalInput")
    corrq_d = nc.dram_tensor("corrq", [2, GD], bf16, kind="ExternalInput")
    corrk_d = nc.dram_tensor("corrk", [2, GD], bf16, kind="ExternalInput")
    corrv_d = nc.dram_tensor("corrv", [2, GD], bf16, kind="ExternalInput")
    yT_d = nc.dram_tensor("yT", [D, L], f32, kind="ExternalOutput")

    def body(ctx, tc):
        import contextlib

        singles = ctx.enter_context(tc.tile_pool(name="singles", bufs=1))
        bigs = ctx.enter_context(tc.tile_pool(name="bigs", bufs=1))
        work = ctx.enter_context(tc.tile_pool(name="work", bufs=3))
        rows = ctx.enter_context(tc.tile_pool(name="rows", bufs=1))

        # ---- weight / correction loads (once per iteration; idempotent) ----
        wq_sb = singles.tile([PART, DC, GD], bf16, tag="wq")
        wk_sb = singles.tile([PART, DC, GD], bf16, tag="wk")
        wv_sb = singles.tile([PART, DC, GD], bf16, tag="wv")
        nc.sync.dma_start(wq_sb[:], wqT_d.ap().rearrange("(c p) i -> p c i", p=PART))
        nc.sync.dma_start(wk_sb[:], wkT_d.ap().rearrange("(c p) i -> p c i", p=PART))
        nc.sync.dma_start(wv_sb[:], wvT_d.ap().rearrange("(c p) i -> p c i", p=PART))
        wo_sb = singles.tile([PART, 2, D], bf16, tag="wo")
        nc.sync.dma_start(wo_sb[:], woT_d.ap().rearrange("(c p) i -> p c i", p=PART))
        corrq_sb = singles.tile([2, GD], bf16, tag="corrq")
        corrk_sb = singles.tile([2, GD], bf16, tag="corrk")
        corrv_sb = singles.tile([2, GD], bf16, tag="corrv")
        nc.sync.dma_start(corrq_sb[:], corrq_d.ap())
        nc.sync.dma_start(corrk_sb[:], corrk_d.ap())
        nc.sync.dma_start(corrv_sb[:], corrv_d.ap())

        ones_col = singles.tile([PART, 1], bf16, tag="ones_col")
        nc.vector.memset(ones_col[:], 1.0)

        # ---- load xT chunks ----
        xt = []
        for kk in range(DC):
            t = bigs.tile([PART, L], bf16, tag=f"xt{kk}", name=f"xt{kk}")
            nc.sync.dma_start(t[:], xT_d.ap()[kk * PART:(kk + 1) * PART, :])
            xt.append(t)

        # ---- LN stats: S1 = sum_d x, S2 = sum_d x^2 (PE ones-matmuls) ----
        stat_scope = contextlib.ExitStack()
        psum_stat = stat_scope.enter_context(
            tc.tile_pool(name="psum_stat", bufs=1, space=bass.MemorySpace.PSUM)
        )
        s1_ps = [psum_stat.tile([1, NB], f32, tag=f"s1_{qc}", name=f"s1_{qc}") for qc in range(4)]
        s2_ps = [psum_stat.tile([1, NB], f32, tag=f"s2_{qc}", name=f"s2_{qc}") for qc in range(4)]
        for kk in range(DC):
            sq = work.tile([PART, L], bf16, tag="sq", bufs=2)
            nc.vector.tensor_mul(sq[:], xt[kk][:], xt[kk][:])
            for qc in range(4):
                sl = slice(qc * NB, (qc + 1) * NB)
                nc.tensor.matmul(
                    s1_ps[qc][:], ones_col[:], xt[kk][:, sl],
                    start=(kk == 0), stop=(kk == DC - 1),
                )
                nc.tensor.matmul(
                    s2_ps[qc][:], ones_col[:], sq[:, sl],
                    start=(kk == 0), stop=(kk == DC - 1),
                )

        # ---- row math: a = rsqrt(var+eps), c = -mean * a  (f32 rows) ----
        m_row = rows.tile([1, L], f32, tag="m_row")
        v_row = rows.tile([1, L], f32, tag="v_row")
        for qc in range(4):
            sl = slice(qc * NB, (qc + 1) * NB)
            nc.vector.tensor_scalar_mul(m_row[:, sl], s1_ps[qc][:], 1.0 / D)
            nc.vector.tensor_scalar_mul(v_row[:, sl], s2_ps[qc][:], 1.0 / D)
        mm_row = rows.tile([1, L], f32, tag="rowscratch")
        nc.vector.tensor_mul(mm_row[:], m_row[:], m_row[:])
        nc.vector.tensor_sub(v_row[:], v_row[:], mm_row[:])
        # sd = sqrt(var + eps); a = 1/sd
        eps_t = rows.tile([1, 1], f32, tag="eps_t")
        nc.vector.memset(eps_t[:], EPS)
        nc.scalar.activation(v_row[:], v_row[:], AF.Sqrt, bias=eps_t[:])
        a32_row = rows.tile([1, L], f32, tag="a32_row")
        nc.vector.reciprocal(a32_row[:], v_row[:])
        a_row = rows.tile([1, L], bf16, tag="a_row")
        nc.vector.tensor_copy(a_row[:], a32_row[:])
        crows = rows.tile([2, L], bf16, tag="crows")
        nc.vector.memset(crows[:], 1.0)   # row 1 stays all-ones
        c32_row = rows.tile([1, L], f32, tag="rowscratch")
        nc.vector.tensor_mul(c32_row[:], m_row[:], a32_row[:])
        nc.vector.tensor_scalar_mul(crows[0:1, :], c32_row[:], -1.0)

        stat_scope.close()

        # broadcast a_row over 128 partitions (SBUF->SBUF DMA, step-0)
        a_bc = bigs.tile([PART, L], bf16, tag="a_bc")
        nc.gpsimd.partition_broadcast(a_bc[:], a_row[:])

        # ---- hTs = xT * a (in place; xt tiles become hTs) ----
        ht = xt
        for kk in range(DC):
            nc.vector.tensor_mul(ht[kk][:], xt[kk][:], a_bc[:])

        # ---- qT / kT projections ([256, L], heads packed 2 per tile) ----
        proj_scope = contextlib.ExitStack()
        psum = proj_scope.enter_context(
            tc.tile_pool(name="psum_proj", bufs=3, space=bass.MemorySpace.PSUM)
        )
        qsb = [bigs.tile([PART, L], bf16, tag=f"q{mc}", name=f"q{mc}") for mc in range(2)]
        ksb = [bigs.tile([PART, L], bf16, tag=f"k{mc}", name=f"k{mc}") for mc in range(2)]
        for (w_sb, corr_sb, dest) in ((wq_sb, corrq_sb, qsb), (wk_sb, corrk_sb, ksb)):
            for mc in range(2):
                msl = slice(mc * PART, (mc + 1) * PART)
                for qc in range(4):
                    sl = slice(qc * NB, (qc + 1) * NB)
                    ps = psum.tile([PART, NB], f32, tag="proj_ps")
                    for kk in range(DC):
                        nc.tensor.matmul(
                            ps[:], w_sb[:, kk, msl], ht[kk][:, sl],
                            start=(kk == 0), stop=False,
                        )
                    nc.tensor.matmul(
                        ps[:], corr_sb[:, msl], crows[:, sl],
                        start=False, stop=True,
                    )
                    nc.vector.tensor_copy(dest[mc][:, sl], ps[:])

        # ---- v projection, natural layout [L, 4, 65] with ones column ----
        vt = []
        for lt in range(LT):
            t = bigs.tile([PART, GH, HD + 1], bf16, tag=f"v{lt}", name=f"v{lt}")
            nc.vector.memset(t[:, :, HD:HD + 1], 1.0)
            vt.append(t)
        for lt in range(LT):
            lsl = slice(lt * PART, (lt + 1) * PART)
            ps = psum.tile([PART, GD], f32, tag="v_ps")
            for kk in range(DC):
                nc.tensor.matmul(
                    ps[:], ht[kk][:, lsl], wv_sb[:, kk, :],
                    start=(kk == 0), stop=False,
                )
            nc.tensor.matmul(ps[:], crows[:, lsl], corrv_sb[:], start=False, stop=True)
            nc.vector.tensor_copy(
                vt[lt][:, :, 0:HD], ps[:].rearrange("p (h d) -> p h d", h=GH)
            )

        proj_scope.close()

        # ---- attention (per head, per q-half) ----
        attn_scope = contextlib.ExitStack()
        psum_ot = attn_scope.enter_context(
            tc.tile_pool(name="psum_ot", bufs=2, space=bass.MemorySpace.PSUM)
        )
        psum_st = attn_scope.enter_context(
            tc.tile_pool(name="psum_st", bufs=4, space=bass.MemorySpace.PSUM)
        )
        opair = [bigs.tile([PART, L], bf16, tag=f"o{mc}", name=f"o{mc}") for mc in range(2)]
        QH = 2             # q processed in halves of 1024
        QW = L // QH
        for h in range(GH):
            hp = h % 2     # position within its kT/qT partition tile
            hm = h // 2    # which kT/qT tile
            psl = slice(hp * HD, (hp + 1) * HD)
            for qh in range(QH):
                otp = [psum_ot.tile([HD + 1, NB], f32, tag=f"otp{i}", name=f"otp{i}") for i in range(2)]
                for kc in range(LT):
                    ksl = slice(kc * PART, (kc + 1) * PART)
                    expst = work.tile([PART, QW], bf16, tag="expst")
                    for half in range(2):
                        qsl = slice(qh * QW + half * NB, qh * QW + (half + 1) * NB)
                        stp = psum_st.tile([PART, NB], f32, tag="stp")
                        nc.tensor.matmul(
                            stp[:], ksb[hm][psl, ksl], qsb[hm][psl, qsl],
                            start=True, stop=True,
                        )
                        nc.scalar.activation(
                            expst[:, half * NB:(half + 1) * NB], stp[:],
                            AF.Exp, scale=float(HD) ** -0.5,
                        )
                    for half in range(2):
                        nc.tensor.matmul(
                            otp[half][:], vt[kc][:, h, :],
                            expst[:, half * NB:(half + 1) * NB],
                            start=(kc == 0), stop=(kc == LT - 1),
                        )
                # normalize + evict
                invd = rows.tile([1, QW], f32, tag="invd")
                for half in range(2):
                    nc.vector.reciprocal(
                        invd[:, half * NB:(half + 1) * NB], otp[half][HD:HD + 1, :]
                    )
                invb = work.tile([HD, QW], f32, tag="invb", bufs=2)
                nc.gpsimd.partition_broadcast(invb[:], invd[:])
                if hp == 0:
                    for half in range(2):
                        nc.vector.tensor_mul(
                            opair[hm][0:HD, qh * QW + half * NB: qh * QW + (half + 1) * NB],
                            otp[half][0:HD, :],
                            invb[:, half * NB:(half + 1) * NB],
                        )
                else:
                    otmp = work.tile([HD, QW], bf16, tag="otmp", bufs=2)
                    for half in range(2):
                        nc.vector.tensor_mul(
                            otmp[:, half * NB:(half + 1) * NB],
                            otp[half][0:HD, :],
                            invb[:, half * NB:(half + 1) * NB],
                        )
                    nc.sync.dma_start(
                        opair[hm][HD:2 * HD, qh * QW:(qh + 1) * QW], otmp[:]
                    )

        attn_scope.close()

        # ---- output projection: yT[d, :] = sum_m woT[m, d] * opair[m] ----
        out_scope = contextlib.ExitStack()
        psum_out = out_scope.enter_context(
            tc.tile_pool(name="psum_out", bufs=3, space=bass.MemorySpace.PSUM)
        )
        for dcix in range(DC):
            dsl = slice(dcix * PART, (dcix + 1) * PART)
            yts = work.tile([PART, L], f32, tag="yts", bufs=2)
            for qc in range(4):
                sl = slice(qc * NB, (qc + 1) * NB)
                ps = psum_out.tile([PART, NB], f32, tag="y_ps")
                for mc in range(2):
                    nc.tensor.matmul(
                        ps[:], wo_sb[:, mc, dsl], opair[mc][:, sl],
                        start=(mc == 0), stop=(mc == 1),
                    )
                nc.vector.tensor_copy(yts[:, sl], ps[:])
            nc.sync.dma_start(yT_d.ap()[dsl, :], yts[:])
        out_scope.close()

    import contextlib

    with tile.TileContext(nc) as tc:
        with contextlib.ExitStack() as ctx:
            if n_iter > 1:
                with tc.For_i(
                    0, n_iter, 1,
                    hint_engines=(EngineType.PE, EngineType.Activation,
                                  EngineType.DVE, EngineType.SP),
                ):
                    with contextlib.ExitStack() as ctx2:
                        body(ctx2, tc)
            else:
                body(ctx, tc)

    nc.compile()
    return nc


def prepare_in_maps(inputs):
    """Host-side sharding / folding. Returns per-core input dicts."""
    x = np.asarray(inputs["x"], np.float32)
    gamma = np.asarray(inputs["ln_gamma"], np.float32)
    beta = np.asarray(inputs["ln_beta"], np.float32)
    Wq = np.asarray(inputs["Wq"], np.float32)
    bq = np.asarray(inputs["bq"], np.float32)
    Wk = np.asarray(inputs["Wk"], np.float32)
    bk = np.asarray(inputs["bk"], np.float32)
    Wv = np.asarray(inputs["Wv"], np.float32)
    bv = np.asarray(inputs["bv"], np.float32)
    Wo = np.asarray(inputs["Wo"], np.float32)

    in_maps = []
    for c in range(N_CORES):
        b, g = divmod(c, HG)
        gsl = slice(g * GD, (g + 1) * GD)
        m = {"xT": np.ascontiguousarray(x[b].T).astype(BF16)}
        for name, W, bias in (("q", Wq, bq), ("k", Wk, bk), ("v", Wv, bv)):
            W_eff = (W * gamma[None, :])[gsl]          # [GD, D]
            b_eff = bias[gsl] + W[gsl] @ beta          # [GD]
            wsum = W_eff.sum(axis=1)                   # [GD]
            m[f"w{name}T"] = np.ascontiguousarray(W_eff.T).astype(BF16)
            m[f"corr{name}"] = np.stack([wsum, b_eff]).astype(BF16)
        m["woT"] = np.ascontiguousarray(Wo[:, gsl].T).astype(BF16)
        in_maps.append(m)
    return in_maps


def gather_output(inputs, results):
    x = np.asarray(inputs["x"], np.float32)
    bo = np.asarray(inputs["bo"], np.float32)
    out = np.empty((B, L, D), np.float32)
    for b in range(B):
        acc = x[b] + bo[None, :]
        for g in range(HG):
            acc = acc + results[b * HG + g]["yT"].T
        out[b] = acc
    return out


_PROGRAM_CACHE = {}


def _get_program(n_iter=1):
    if n_iter not in _PROGRAM_CACHE:
        _PROGRAM_CACHE[n_iter] = _build_program(n_iter)
    return _PROGRAM_CACHE[n_iter]


def kernel(**inputs):
    from concourse import bass_utils

    nc = _get_program(1)
    in_maps = prepare_in_maps(inputs)
    res = bass_utils.run_bass_kernel_spmd(nc, in_maps, core_ids=list(range(N_CORES)))
    return gather_output(inputs, res.results)


# revision 24
# speedup vs baseline: 10.6462x; 10.6462x over previous
"""Trainium2 Bass kernel for a pre-LN attention block.

Reference computation (B=2, L=2048, D=1024, H=16, hd=64):
    h = LayerNorm(x) * gamma + beta
    q, k, v = h @ W{q,k,v}.T + b{q,k,v}      (split into 16 heads of 64)
    o = softmax(q k^T / sqrt(hd)) v
    out = x + (o @ Wo.T + bo)

Sharding over 8 cores: core c handles batch b = c // 4 and head group
g = c % 4 (4 heads, 256 hidden dims).  Each core computes a partial
output  Ypart = attn_heads_g(LN(x[b])) @ Wo[:, g]T ; the host sums the
four partials per batch and adds the residual and bo in fp32.

Device-side layout is fully "feature-on-partitions" (transposed):
  - xT [D, L] bf16 arrives pre-transposed from host.
  - LN row stats (mean, rsqrt(var)) are computed with ones-matmuls on
    the tensor engine; normalization is hTs = xT * a_row (a = rsqrt),
    while the mean term (and LN beta / projection biases, gamma is
    folded into the weights host-side) enters each projection as a K=2
    correction matmul accumulated into the same PSUM group.
  - qT/kT [256, L]; v in natural layout [L, 4, 65] with a ones column.
  - Per head: ST = k q^T -> PSUM; exp(ST/8) is the PSUM->SBUF eviction
    on the scalar engine; OT' = [V|1]^T @ exp(ST) gives both the head
    output and the softmax denominator (row 64); normalization happens
    during OT' eviction via a broadcast reciprocal row.
"""

import numpy as np
import ml_dtypes

BF16 = ml_dtypes.bfloat16

B, L, D = 2, 2048, 1024
H, HD = 16, 64
HG = 4                 # head groups (cores per batch)
GH = H // HG           # heads per group = 4
GD = GH * HD           # hidden dims per group = 256
N_CORES = 8
PART = 128
NB = 512               # matmul moving free dim / PSUM bank width (fp32)
DC = D // PART         # 8 chunks of the contraction dim
LT = L // PART         # 16 L-tiles
EPS = 1e-5


def _build_program(n_iter: int = 1, phases: int = 4):
    """Build + compile the SPMD single-core program. n_iter > 1 wraps the
    whole computation in a hardware loop (for slope-based timing)."""
    import concourse.bass as bass
    import concourse.bacc as bacc
    import concourse.tile as tile
    import concourse.mybir as mybir
    from concourse.engine_type import EngineType

    f32 = mybir.dt.float32
    bf16 = mybir.dt.bfloat16
    AF = mybir.ActivationFunctionType

    nc = bacc.Bacc("TRN2", target_bir_lowering=False, debug=False)

    xT_d = nc.dram_tensor("xT", [D, L], bf16, kind="ExternalInput")
    wqT_d = nc.dram_tensor("wqT", [D, GD], bf16, kind="ExternalInput")
    wkT_d = nc.dram_tensor("wkT", [D, GD], bf16, kind="ExternalInput")
    wvT_d = nc.dram_tensor("wvT", [D, GD], bf16, kind="ExternalInput")
    woT_d = nc.dram_tensor("woT", [GD, D], bf16, kind="ExternalInput")
    corrq_d = nc.dram_tensor("corrq", [2, GD], bf16, kind="ExternalInput")
    corrk_d = nc.dram_tensor("corrk", [2, GD], bf16, kind="ExternalInput")
    corrv_d = nc.dram_tensor("corrv", [2, GD], bf16, kind="ExternalInput")
    yT_d = nc.dram_tensor("yT", [D, L], bf16, kind="ExternalOutput")

    def body(ctx, tc, phases=4):
        import contextlib

        singles = ctx.enter_context(tc.tile_pool(name="singles", bufs=1))
        bigs = ctx.enter_context(tc.tile_pool(name="bigs", bufs=1))
        work = ctx.enter_context(tc.tile_pool(name="work", bufs=3))
        rows = ctx.enter_context(tc.tile_pool(name="rows", bufs=1))

        # ---- load xT chunks first (stats are the critical path) ----
        xt = []
        for kk in range(DC):
            t = bigs.tile([PART, L], bf16, tag=f"xt{kk}", name=f"xt{kk}")
            nc.sync.dma_start(t[:], xT_d.ap()[kk * PART:(kk + 1) * PART, :])
            xt.append(t)

        # ---- weight / correction loads (once per iteration; idempotent) ----
        wq_sb = singles.tile([PART, DC, GD], bf16, tag="wq")
        wk_sb = singles.tile([PART, DC, GD], bf16, tag="wk")
        wv_sb = singles.tile([PART, DC, GD], bf16, tag="wv")
        nc.sync.dma_start(wq_sb[:], wqT_d.ap().rearrange("(c p) i -> p c i", p=PART))
        nc.sync.dma_start(wk_sb[:], wkT_d.ap().rearrange("(c p) i -> p c i", p=PART))
        nc.sync.dma_start(wv_sb[:], wvT_d.ap().rearrange("(c p) i -> p c i", p=PART))
        wo_sb = singles.tile([PART, 2, D], bf16, tag="wo")
        nc.sync.dma_start(wo_sb[:], woT_d.ap().rearrange("(c p) i -> p c i", p=PART))
        corrq_sb = singles.tile([2, GD], bf16, tag="corrq")
        corrk_sb = singles.tile([2, GD], bf16, tag="corrk")
        corrv_sb = singles.tile([2, GD], bf16, tag="corrv")
        nc.sync.dma_start(corrq_sb[:], corrq_d.ap())
        nc.sync.dma_start(corrk_sb[:], corrk_d.ap())
        nc.sync.dma_start(corrv_sb[:], corrv_d.ap())

        ones_col = singles.tile([PART, 1], bf16, tag="ones_col")
        nc.vector.memset(ones_col[:], 1.0)

        # ---- LN stats: S1 = sum_d x, S2 = sum_d x^2 (PE ones-matmuls) ----
        stat_scope = contextlib.ExitStack()
        psum_stat = stat_scope.enter_context(
            tc.tile_pool(name="psum_stat", bufs=1, space=bass.MemorySpace.PSUM)
        )
        s1_ps = [psum_stat.tile([1, NB], f32, tag=f"s1_{qc}", name=f"s1_{qc}") for qc in range(4)]
        s2_ps = [psum_stat.tile([1, NB], f32, tag=f"s2_{qc}", name=f"s2_{qc}") for qc in range(4)]
        for kk in range(DC):
            sq = work.tile([PART, L], bf16, tag="sq", bufs=2)
            nc.vector.tensor_mul(sq[:], xt[kk][:], xt[kk][:])
            for qc in range(4):
                sl = slice(qc * NB, (qc + 1) * NB)
                nc.tensor.matmul(
                    s1_ps[qc][:], ones_col[:], xt[kk][:, sl],
                    start=(kk == 0), stop=(kk == DC - 1),
                )
                nc.tensor.matmul(
                    s2_ps[qc][:], ones_col[:], sq[:, sl],
                    start=(kk == 0), stop=(kk == DC - 1),
                )

        # ---- row math: a = rsqrt(var+eps), c = -mean * a  (f32 rows) ----
        m_row = rows.tile([1, L], f32, tag="m_row")
        v_row = rows.tile([1, L], f32, tag="v_row")
        for qc in range(4):
            sl = slice(qc * NB, (qc + 1) * NB)
            nc.vector.tensor_scalar_mul(m_row[:, sl], s1_ps[qc][:], 1.0 / D)
            nc.vector.tensor_scalar_mul(v_row[:, sl], s2_ps[qc][:], 1.0 / D)
        mm_row = rows.tile([1, L], f32, tag="rowscratch")
        nc.vector.tensor_mul(mm_row[:], m_row[:], m_row[:])
        nc.vector.tensor_sub(v_row[:], v_row[:], mm_row[:])
        # sd = sqrt(var + eps); a = 1/sd
        eps_t = rows.tile([1, 1], f32, tag="eps_t")
        nc.vector.memset(eps_t[:], EPS)
        nc.scalar.activation(v_row[:], v_row[:], AF.Sqrt, bias=eps_t[:])
        a32_row = rows.tile([1, L], f32, tag="a32_row")
        nc.vector.reciprocal(a32_row[:], v_row[:])
        a_row = rows.tile([1, L], bf16, tag="a_row")
        nc.vector.tensor_copy(a_row[:], a32_row[:])
        crows = rows.tile([2, L], bf16, tag="crows")
        nc.vector.memset(crows[:], 1.0)   # row 1 stays all-ones
        c32_row = rows.tile([1, L], f32, tag="rowscratch")
        nc.vector.tensor_mul(c32_row[:], m_row[:], a32_row[:])
        nc.vector.tensor_scalar_mul(crows[0:1, :], c32_row[:], -1.0)

        stat_scope.close()

        # broadcast a_row over 128 partitions (SBUF->SBUF DMA, step-0)
        a_bc = bigs.tile([PART, L], bf16, tag="a_bc")
        nc.gpsimd.partition_broadcast(a_bc[:], a_row[:])

        # ---- hTs = xT * a (in place; xt tiles become hTs) ----
        ht = xt
        for kk in range(DC):
            nc.vector.tensor_mul(ht[kk][:], xt[kk][:], a_bc[:])

        if phases < 2:
            return
        # ---- qT / kT projections ([256, L], heads packed 2 per tile) ----
        proj_scope = contextlib.ExitStack()
        psum = proj_scope.enter_context(
            tc.tile_pool(name="psum_proj", bufs=3, space=bass.MemorySpace.PSUM)
        )
        qsb = [bigs.tile([PART, L], bf16, tag=f"q{mc}", name=f"q{mc}") for mc in range(2)]
        ksb = [bigs.tile([PART, L], bf16, tag=f"k{mc}", name=f"k{mc}") for mc in range(2)]
        for (w_sb, corr_sb, dest) in ((wq_sb, corrq_sb, qsb), (wk_sb, corrk_sb, ksb)):
            for mc in range(2):
                msl = slice(mc * PART, (mc + 1) * PART)
                for qc in range(4):
                    sl = slice(qc * NB, (qc + 1) * NB)
                    ps = psum.tile([PART, NB], f32, tag="proj_ps")
                    for kk in range(DC):
                        nc.tensor.matmul(
                            ps[:], w_sb[:, kk, msl], ht[kk][:, sl],
                            start=(kk == 0), stop=False,
                        )
                    nc.tensor.matmul(
                        ps[:], corr_sb[:, msl], crows[:, sl],
                        start=False, stop=True,
                    )
                    nc.vector.tensor_copy(dest[mc][:, sl], ps[:])

        # ---- v projection, natural layout [L, 4, 65]; col 64 = ones so the
        # PV matmul also accumulates the softmax denominator into row 64 ----
        vt = []
        for lt in range(LT):
            t = bigs.tile([PART, GH, HD + 1], bf16, tag=f"v{lt}", name=f"v{lt}")
            nc.vector.memset(t[:, :, HD:HD + 1], 1.0)
            vt.append(t)
        for lt in range(LT):
            lsl = slice(lt * PART, (lt + 1) * PART)
            ps = psum.tile([PART, GD], f32, tag="v_ps")
            for kk in range(DC):
                nc.tensor.matmul(
                    ps[:], ht[kk][:, lsl], wv_sb[:, kk, :],
                    start=(kk == 0), stop=False,
                )
            nc.tensor.matmul(
                ps[:], crows[0:1, lsl], corrv_sb[0:1, :], start=False, stop=True
            )
            nc.vector.tensor_copy(
                vt[lt][:, :, 0:HD], ps[:].rearrange("p (h d) -> p h d", h=GH)
            )

        proj_scope.close()

        if phases < 3:
            return
        # ---- attention: head pairs, row-packed QK, col-packed PV ----
        attn_scope = contextlib.ExitStack()
        psum_ot = attn_scope.enter_context(
            tc.tile_pool(name="psum_ot", bufs=2, space=bass.MemorySpace.PSUM)
        )
        psum_st = attn_scope.enter_context(
            tc.tile_pool(name="psum_st", bufs=2, space=bass.MemorySpace.PSUM)
        )
        dram_scr = attn_scope.enter_context(
            tc.tile_pool(name="dram_scr", bufs=2, space="DRAM")
        )
        opair = [bigs.tile([PART, L], bf16, tag=f"o{mc}", name=f"o{mc}") for mc in range(2)]
        QW = 512
        NQH = L // QW
        for pm in range(2):            # pair pm handles heads (2pm, 2pm+1)
            for qh in range(NQH):
                qsl = slice(qh * QW, (qh + 1) * QW)
                # per-head PSUM accumulators [65, QW]: rows 0-63 = O^T,
                # row 64 = softmax denominator (from V's ones column)
                otp = [
                    psum_ot.tile([HD + 1, QW], f32, tag=f"otp{i}", name=f"otp{i}")
                    for i in range(2)
                ]
                nc.vector.memset(otp[0][:], 0.0)
                nc.vector.memset(otp[1][:], 0.0)

                # software-pipelined emission: QK(kc+1) is emitted before
                # PV(kc) so the PE stream overlaps with the exp eviction.
                def emit_qk(kc):
                    ksl = slice(kc * PART, (kc + 1) * PART)
                    stp = psum_st.tile([PART, 2 * QW], f32, tag="stp", name="stp")
                    for hp in range(2):
                        psl = slice(hp * HD, (hp + 1) * HD)
                        nc.tensor.matmul(
                            stp[:, hp * QW:(hp + 1) * QW],
                            ksb[pm][psl, ksl], qsb[pm][psl, qsl],
                            start=True, stop=True,
                        )
                    return stp

                def emit_exp(stp):
                    expst = work.tile([PART, 2 * QW], bf16, tag="expst", bufs=3)
                    nc.scalar.activation(
                        expst[:], stp[:], AF.Exp, scale=float(HD) ** -0.5
                    )
                    return expst

                def emit_pv(kc, expst):
                    # 4 half-K matmuls as two concurrent row-group pairs:
                    # (h0 rows 0-63, h1 rows 64-127), (h1 rows 0-63, h0 rows 64-127)
                    last = kc == LT - 1
                    for hp in range(2):
                        nc.tensor.matmul(
                            otp[hp][:],
                            vt[kc][:, 2 * pm + hp, :],
                            expst[:, hp * QW:(hp + 1) * QW],
                            start=False, stop=last,
                            skip_group_check=True,
                        )

                stp_cur = emit_qk(0)
                for kc in range(LT):
                    expst_cur = emit_exp(stp_cur)
                    if kc + 1 < LT:
                        stp_cur = emit_qk(kc + 1)
                    emit_pv(kc, expst_cur)

                # normalize + evict; reciprocal rows bounce through DRAM
                # (partition_broadcast on HW only supports base0->base0)
                invd = rows.tile([HD + 1, QW], f32, tag="invd", name="invd")
                nc.vector.reciprocal(invd[HD:HD + 1, :], otp[0][HD:HD + 1, :])
                dscr = dram_scr.tile([2, QW], f32, tag="dscr", bufs=2)
                nc.sync.dma_start(dscr[0:1, :], invd[HD:HD + 1, :])
                invd2 = rows.tile([HD + 1, QW], f32, tag="invd2", name="invd2")
                nc.vector.reciprocal(invd2[HD:HD + 1, :], otp[1][HD:HD + 1, :])
                nc.sync.dma_start(dscr[1:2, :], invd2[HD:HD + 1, :])
                invb = [
                    work.tile([HD, QW], f32, tag=f"invb{i}", bufs=2, name=f"invb{i}")
                    for i in range(2)
                ]
                for hp in range(2):
                    row = dscr[hp:hp + 1, :]
                    bc_src = bass.AP(
                        tensor=row.tensor, offset=row.offset,
                        ap=[[0, HD]] + [list(d) for d in row.ap[1:]],
                    )
                    nc.gpsimd.dma_start(invb[hp][:], bc_src)
                # head even: direct evict; head odd: via otmp + partition shift DMA
                nc.vector.tensor_mul(opair[pm][0:HD, qsl], otp[0][0:HD, :], invb[0][:])
                otmp = work.tile([HD, QW], bf16, tag="otmp", bufs=2)
                nc.vector.tensor_mul(otmp[:], otp[1][0:HD, :], invb[1][:])
                nc.sync.dma_start(opair[pm][HD:2 * HD, qsl], otmp[:])

        attn_scope.close()
        if phases < 4:
            return

        # ---- output projection: yT[d, :] = sum_m woT[m, d] * opair[m] ----
        out_scope = contextlib.ExitStack()
        psum_out = out_scope.enter_context(
            tc.tile_pool(name="psum_out", bufs=3, space=bass.MemorySpace.PSUM)
        )
        for dcix in range(DC):
            dsl = slice(dcix * PART, (dcix + 1) * PART)
            yts = work.tile([PART, L], bf16, tag="yts", bufs=2)
            for qc in range(4):
                sl = slice(qc * NB, (qc + 1) * NB)
                ps = psum_out.tile([PART, NB], f32, tag="y_ps")
                for mc in range(2):
                    nc.tensor.matmul(
                        ps[:], wo_sb[:, mc, dsl], opair[mc][:, sl],
                        start=(mc == 0), stop=(mc == 1),
                    )
                nc.vector.tensor_copy(yts[:, sl], ps[:])
            nc.sync.dma_start(yT_d.ap()[dsl, :], yts[:])
        out_scope.close()

    import contextlib

    with tile.TileContext(nc) as tc:
        with contextlib.ExitStack() as ctx:
            if n_iter > 1:
                with tc.For_i(
                    0, n_iter, 1,
                    hint_engines=(EngineType.PE, EngineType.Activation,
                                  EngineType.DVE, EngineType.SP),
                ):
                    with contextlib.ExitStack() as ctx2:
                        body(ctx2, tc, phases)
            else:
                body(ctx, tc, phases)

    nc.compile()
    return nc


def prepare_in_maps(inputs):
    """Host-side sharding / folding. Returns per-core input dicts."""
    x = np.asarray(inputs["x"], np.float32)
    gamma = np.asarray(inputs["ln_gamma"], np.float32)
    beta = np.asarray(inputs["ln_beta"], np.float32)
    Wq = np.asarray(inputs["Wq"], np.float32)
    bq = np.asarray(inputs["bq"], np.float32)
    Wk = np.asarray(inputs["Wk"], np.float32)
    bk = np.asarray(inputs["bk"], np.float32)
    Wv = np.asarray(inputs["Wv"], np.float32)
    bv = np.asarray(inputs["bv"], np.float32)
    Wo = np.asarray(inputs["Wo"], np.float32)

    in_maps = []
    for c in range(N_CORES):
        b, g = divmod(c, HG)
        gsl = slice(g * GD, (g + 1) * GD)
        m = {"xT": np.ascontiguousarray(x[b].T).astype(BF16)}
        for name, W, bias in (("q", Wq, bq), ("k", Wk, bk), ("v", Wv, bv)):
            W_eff = (W * gamma[None, :])[gsl]          # [GD, D]
            if name == "v":
                # bv and the beta contribution pass through softmax-normalized
                # attention as a constant row; both fold into bo on the host
                # (see gather_output). Device v needs only the mean term.
                b_eff = np.zeros(GD, np.float32)
            else:
                b_eff = bias[gsl] + W[gsl] @ beta      # [GD]
            wsum = W_eff.sum(axis=1)                   # [GD]
            m[f"w{name}T"] = np.ascontiguousarray(W_eff.T).astype(BF16)
            m[f"corr{name}"] = np.stack([wsum, b_eff]).astype(BF16)
        m["woT"] = np.ascontiguousarray(Wo[:, gsl].T).astype(BF16)
        in_maps.append(m)
    return in_maps


def gather_output(inputs, results):
    x = np.asarray(inputs["x"], np.float32)
    # bv (and beta's contribution through Wv) shift every value row by a
    # constant; softmax rows sum to 1, so the attention output shifts by that
    # same constant and bo absorbs it exactly: bo_eff = bo + Wo @ bv_eff.
    bv_eff = np.asarray(inputs["bv"], np.float32) + (
        np.asarray(inputs["Wv"], np.float32) @ np.asarray(inputs["ln_beta"], np.float32)
    )
    bo = np.asarray(inputs["bo"], np.float32) + (
        np.asarray(inputs["Wo"], np.float32) @ bv_eff
    )
    out = np.empty((B, L, D), np.float32)
    for b in range(B):
        acc = x[b] + bo[None, :]
        for g in range(HG):
            acc = acc + results[b * HG + g]["yT"].astype(np.float32).T
        out[b] = acc
    return out


_PROGRAM_CACHE = {}


def _get_program(n_iter=1, phases=4):
    key = (n_iter, phases)
    if key not in _PROGRAM_CACHE:
        _PROGRAM_CACHE[key] = _build_program(n_iter, phases)
    return _PROGRAM_CACHE[key]


def kernel(**inputs):
    from concourse import bass_utils

    nc = _get_program(1)
    in_maps = prepare_in_maps(inputs)
    res = bass_utils.run_bass_kernel_spmd(nc, in_maps, core_ids=list(range(N_CORES)))
    return gather_output(inputs, res.results)


# revision 26
# speedup vs baseline: 11.2521x; 1.0569x over previous
"""Trainium2 Bass kernel for a pre-LN attention block.

Reference computation (B=2, L=2048, D=1024, H=16, hd=64):
    h = LayerNorm(x) * gamma + beta
    q, k, v = h @ W{q,k,v}.T + b{q,k,v}      (split into 16 heads of 64)
    o = softmax(q k^T / sqrt(hd)) v
    out = x + (o @ Wo.T + bo)

Sharding over 8 cores: core c handles batch b = c // 4 and head group
g = c % 4 (4 heads, 256 hidden dims).  Each core computes a partial
output  Ypart = attn_heads_g(LN(x[b])) @ Wo[:, g]T ; the host sums the
four partials per batch and adds the residual and bo in fp32.

Device-side layout is fully "feature-on-partitions" (transposed):
  - xT [D, L] bf16 arrives pre-transposed from host.
  - LN row stats (mean, rsqrt(var)) are computed with ones-matmuls on
    the tensor engine; normalization is hTs = xT * a_row (a = rsqrt),
    while the mean term (and LN beta / projection biases, gamma is
    folded into the weights host-side) enters each projection as a K=2
    correction matmul accumulated into the same PSUM group.
  - qT/kT [256, L]; v in natural layout [L, 4, 65] with a ones column.
  - Per head: ST = k q^T -> PSUM; exp(ST/8) is the PSUM->SBUF eviction
    on the scalar engine; OT' = [V|1]^T @ exp(ST) gives both the head
    output and the softmax denominator (row 64); normalization happens
    during OT' eviction via a broadcast reciprocal row.
"""

import numpy as np
import ml_dtypes

BF16 = ml_dtypes.bfloat16

B, L, D = 2, 2048, 1024
H, HD = 16, 64
HG = 4                 # head groups (cores per batch)
GH = H // HG           # heads per group = 4
GD = GH * HD           # hidden dims per group = 256
N_CORES = 8
PART = 128
NB = 512               # matmul moving free dim / PSUM bank width (fp32)
DC = D // PART         # 8 chunks of the contraction dim
LT = L // PART         # 16 L-tiles
EPS = 1e-5


def _build_program(n_iter: int = 1, phases: int = 4):
    """Build + compile the SPMD single-core program. n_iter > 1 wraps the
    whole computation in a hardware loop (for slope-based timing)."""
    import concourse.bass as bass
    import concourse.bacc as bacc
    import concourse.tile as tile
    import concourse.mybir as mybir
    from concourse.engine_type import EngineType

    f32 = mybir.dt.float32
    bf16 = mybir.dt.bfloat16
    AF = mybir.ActivationFunctionType

    nc = bacc.Bacc("TRN2", target_bir_lowering=False, debug=False)

    xT_d = nc.dram_tensor("xT", [D, L], bf16, kind="ExternalInput")
    wqT_d = nc.dram_tensor("wqT", [D, GD], bf16, kind="ExternalInput")
    wkT_d = nc.dram_tensor("wkT", [D, GD], bf16, kind="ExternalInput")
    wvT_d = nc.dram_tensor("wvT", [D, GD], bf16, kind="ExternalInput")
    woT_d = nc.dram_tensor("woT", [GD, D], bf16, kind="ExternalInput")
    corrq_d = nc.dram_tensor("corrq", [2, GD], bf16, kind="ExternalInput")
    corrk_d = nc.dram_tensor("corrk", [2, GD], bf16, kind="ExternalInput")
    corrv_d = nc.dram_tensor("corrv", [2, GD], bf16, kind="ExternalInput")
    yT_d = nc.dram_tensor("yT", [D, L], bf16, kind="ExternalOutput")

    def body(ctx, tc, phases=4):
        import contextlib

        singles = ctx.enter_context(tc.tile_pool(name="singles", bufs=1))
        bigs = ctx.enter_context(tc.tile_pool(name="bigs", bufs=1))
        work = ctx.enter_context(tc.tile_pool(name="work", bufs=3))
        rows = ctx.enter_context(tc.tile_pool(name="rows", bufs=1))

        # ---- load xT chunks first (stats are the critical path) ----
        xt = []
        for kk in range(DC):
            t = bigs.tile([PART, L], bf16, tag=f"xt{kk}", name=f"xt{kk}")
            nc.sync.dma_start(t[:], xT_d.ap()[kk * PART:(kk + 1) * PART, :])
            xt.append(t)

        # ---- weight / correction loads (once per iteration; idempotent) ----
        wq_sb = singles.tile([PART, DC, GD], bf16, tag="wq")
        wk_sb = singles.tile([PART, DC, GD], bf16, tag="wk")
        wv_sb = singles.tile([PART, DC, GD], bf16, tag="wv")
        nc.sync.dma_start(wq_sb[:], wqT_d.ap().rearrange("(c p) i -> p c i", p=PART))
        nc.sync.dma_start(wk_sb[:], wkT_d.ap().rearrange("(c p) i -> p c i", p=PART))
        nc.sync.dma_start(wv_sb[:], wvT_d.ap().rearrange("(c p) i -> p c i", p=PART))
        wo_sb = singles.tile([PART, 2, D], bf16, tag="wo")
        nc.sync.dma_start(wo_sb[:], woT_d.ap().rearrange("(c p) i -> p c i", p=PART))
        corrq_sb = singles.tile([2, GD], bf16, tag="corrq")
        corrk_sb = singles.tile([2, GD], bf16, tag="corrk")
        corrv_sb = singles.tile([2, GD], bf16, tag="corrv")
        nc.sync.dma_start(corrq_sb[:], corrq_d.ap())
        nc.sync.dma_start(corrk_sb[:], corrk_d.ap())
        nc.sync.dma_start(corrv_sb[:], corrv_d.ap())

        ones_col = singles.tile([PART, 1], bf16, tag="ones_col")
        nc.vector.memset(ones_col[:], 1.0)

        # ---- LN stats: S1 = sum_d x, S2 = sum_d x^2 (PE ones-matmuls) ----
        stat_scope = contextlib.ExitStack()
        psum_stat = stat_scope.enter_context(
            tc.tile_pool(name="psum_stat", bufs=1, space=bass.MemorySpace.PSUM)
        )
        s1_ps = [psum_stat.tile([1, NB], f32, tag=f"s1_{qc}", name=f"s1_{qc}") for qc in range(4)]
        s2_ps = [psum_stat.tile([1, NB], f32, tag=f"s2_{qc}", name=f"s2_{qc}") for qc in range(4)]
        for kk in range(DC):
            sq = work.tile([PART, L], bf16, tag="sq", bufs=2)
            nc.vector.tensor_mul(sq[:], xt[kk][:], xt[kk][:])
            for qc in range(4):
                sl = slice(qc * NB, (qc + 1) * NB)
                nc.tensor.matmul(
                    s1_ps[qc][:], ones_col[:], xt[kk][:, sl],
                    start=(kk == 0), stop=(kk == DC - 1),
                )
                nc.tensor.matmul(
                    s2_ps[qc][:], ones_col[:], sq[:, sl],
                    start=(kk == 0), stop=(kk == DC - 1),
                )

        # ---- row math: a = rsqrt(var+eps), c = -mean * a  (f32 rows) ----
        m_row = rows.tile([1, L], f32, tag="m_row")
        v_row = rows.tile([1, L], f32, tag="v_row")
        for qc in range(4):
            sl = slice(qc * NB, (qc + 1) * NB)
            nc.vector.tensor_scalar_mul(m_row[:, sl], s1_ps[qc][:], 1.0 / D)
            nc.vector.tensor_scalar_mul(v_row[:, sl], s2_ps[qc][:], 1.0 / D)
        mm_row = rows.tile([1, L], f32, tag="rowscratch")
        nc.vector.tensor_mul(mm_row[:], m_row[:], m_row[:])
        nc.vector.tensor_sub(v_row[:], v_row[:], mm_row[:])
        # sd = sqrt(var + eps); a = 1/sd
        eps_t = rows.tile([1, 1], f32, tag="eps_t")
        nc.vector.memset(eps_t[:], EPS)
        nc.scalar.activation(v_row[:], v_row[:], AF.Sqrt, bias=eps_t[:])
        a32_row = rows.tile([1, L], f32, tag="a32_row")
        nc.vector.reciprocal(a32_row[:], v_row[:])
        a_row = rows.tile([1, L], bf16, tag="a_row")
        nc.vector.tensor_copy(a_row[:], a32_row[:])
        crows = rows.tile([2, L], bf16, tag="crows")
        nc.vector.memset(crows[:], 1.0)   # row 1 stays all-ones
        c32_row = rows.tile([1, L], f32, tag="rowscratch")
        nc.vector.tensor_mul(c32_row[:], m_row[:], a32_row[:])
        nc.vector.tensor_scalar_mul(crows[0:1, :], c32_row[:], -1.0)

        stat_scope.close()

        # broadcast a_row over 128 partitions (SBUF->SBUF DMA, step-0)
        a_bc = bigs.tile([PART, L], bf16, tag="a_bc")
        nc.gpsimd.partition_broadcast(a_bc[:], a_row[:])

        # ---- hTs = xT * a (in place; xt tiles become hTs) ----
        ht = xt
        for kk in range(DC):
            nc.vector.tensor_mul(ht[kk][:], xt[kk][:], a_bc[:])

        if phases < 2:
            return
        # ---- qT / kT projections ([256, L], heads packed 2 per tile) ----
        proj_scope = contextlib.ExitStack()
        psum = proj_scope.enter_context(
            tc.tile_pool(name="psum_proj", bufs=3, space=bass.MemorySpace.PSUM)
        )
        qsb = [bigs.tile([PART, L], bf16, tag=f"q{mc}", name=f"q{mc}") for mc in range(2)]
        ksb = [bigs.tile([PART, L], bf16, tag=f"k{mc}", name=f"k{mc}") for mc in range(2)]
        for (w_sb, corr_sb, dest) in ((wq_sb, corrq_sb, qsb), (wk_sb, corrk_sb, ksb)):
            for mc in range(2):
                msl = slice(mc * PART, (mc + 1) * PART)
                for qc in range(4):
                    sl = slice(qc * NB, (qc + 1) * NB)
                    ps = psum.tile([PART, NB], f32, tag="proj_ps")
                    for kk in range(DC):
                        nc.tensor.matmul(
                            ps[:], w_sb[:, kk, msl], ht[kk][:, sl],
                            start=(kk == 0), stop=False,
                        )
                    nc.tensor.matmul(
                        ps[:], corr_sb[:, msl], crows[:, sl],
                        start=False, stop=True,
                    )
                    nc.vector.tensor_copy(dest[mc][:, sl], ps[:])

        # ---- v projection, natural layout [L, 4, 65]; col 64 = ones so the
        # PV matmul also accumulates the softmax denominator into row 64 ----
        vt = []
        for lt in range(LT):
            t = bigs.tile([PART, GH, HD + 1], bf16, tag=f"v{lt}", name=f"v{lt}")
            nc.vector.memset(t[:, :, HD:HD + 1], 1.0)
            vt.append(t)
        for lt in range(LT):
            lsl = slice(lt * PART, (lt + 1) * PART)
            ps = psum.tile([PART, GD], f32, tag="v_ps")
            for kk in range(DC):
                nc.tensor.matmul(
                    ps[:], ht[kk][:, lsl], wv_sb[:, kk, :],
                    start=(kk == 0), stop=False,
                )
            nc.tensor.matmul(
                ps[:], crows[0:1, lsl], corrv_sb[0:1, :], start=False, stop=True
            )
            nc.vector.tensor_copy(
                vt[lt][:, :, 0:HD], ps[:].rearrange("p (h d) -> p h d", h=GH)
            )

        proj_scope.close()

        if phases < 3:
            return
        # ---- attention: head pairs, row-packed QK, col-packed PV ----
        attn_scope = contextlib.ExitStack()
        psum_ot = attn_scope.enter_context(
            tc.tile_pool(name="psum_ot", bufs=2, space=bass.MemorySpace.PSUM)
        )
        psum_st = attn_scope.enter_context(
            tc.tile_pool(name="psum_st", bufs=2, space=bass.MemorySpace.PSUM)
        )
        dram_scr = attn_scope.enter_context(
            tc.tile_pool(name="dram_scr", bufs=2, space="DRAM")
        )
        opair = [bigs.tile([PART, L], bf16, tag=f"o{mc}", name=f"o{mc}") for mc in range(2)]
        QW = 512
        NQH = L // QW
        for pm in range(2):            # pair pm handles heads (2pm, 2pm+1)
            for qh in range(NQH):
                qsl = slice(qh * QW, (qh + 1) * QW)
                # per-head PSUM accumulators [65, QW]: rows 0-63 = O^T,
                # row 64 = softmax denominator (from V's ones column)
                otp = [
                    psum_ot.tile([HD + 1, QW], f32, tag=f"otp{i}", name=f"otp{i}")
                    for i in range(2)
                ]
                nc.vector.memset(otp[0][:], 0.0)
                nc.vector.memset(otp[1][:], 0.0)

                # software-pipelined emission: QK(kc+1) is emitted before
                # PV(kc) so the PE stream overlaps with the exp eviction.
                def emit_qk(kc):
                    ksl = slice(kc * PART, (kc + 1) * PART)
                    stp = psum_st.tile([PART, 2 * QW], f32, tag="stp", name="stp")
                    for hp in range(2):
                        psl = slice(hp * HD, (hp + 1) * HD)
                        nc.tensor.matmul(
                            stp[:, hp * QW:(hp + 1) * QW],
                            ksb[pm][psl, ksl], qsb[pm][psl, qsl],
                            start=True, stop=True,
                        )
                    return stp

                def emit_exp(stp):
                    expst = work.tile([PART, 2 * QW], bf16, tag="expst", bufs=3)
                    nc.scalar.activation(
                        expst[:], stp[:], AF.Exp, scale=float(HD) ** -0.5
                    )
                    return expst

                def emit_pv(kc, expst):
                    # 4 half-K matmuls as two concurrent row-group pairs:
                    # (h0 rows 0-63, h1 rows 64-127), (h1 rows 0-63, h0 rows 64-127)
                    last = kc == LT - 1
                    for hp in range(2):
                        nc.tensor.matmul(
                            otp[hp][:],
                            vt[kc][:, 2 * pm + hp, :],
                            expst[:, hp * QW:(hp + 1) * QW],
                            start=False, stop=last,
                            skip_group_check=True,
                        )

                stp_cur = emit_qk(0)
                for kc in range(LT):
                    expst_cur = emit_exp(stp_cur)
                    if kc + 1 < LT:
                        stp_cur = emit_qk(kc + 1)
                    emit_pv(kc, expst_cur)

                # normalize + evict; reciprocal rows bounce through DRAM
                # (partition_broadcast on HW only supports base0->base0)
                invd = rows.tile([HD + 1, QW], f32, tag="invd", name="invd")
                nc.vector.reciprocal(invd[HD:HD + 1, :], otp[0][HD:HD + 1, :])
                dscr = dram_scr.tile([2, QW], f32, tag="dscr", bufs=2)
                nc.sync.dma_start(dscr[0:1, :], invd[HD:HD + 1, :])
                invd2 = rows.tile([HD + 1, QW], f32, tag="invd2", name="invd2")
                nc.vector.reciprocal(invd2[HD:HD + 1, :], otp[1][HD:HD + 1, :])
                nc.sync.dma_start(dscr[1:2, :], invd2[HD:HD + 1, :])
                invb = [
                    work.tile([HD, QW], f32, tag=f"invb{i}", bufs=2, name=f"invb{i}")
                    for i in range(2)
                ]
                for hp in range(2):
                    row = dscr[hp:hp + 1, :]
                    bc_src = bass.AP(
                        tensor=row.tensor, offset=row.offset,
                        ap=[[0, HD]] + [list(d) for d in row.ap[1:]],
                    )
                    nc.gpsimd.dma_start(invb[hp][:], bc_src)
                # head even: direct evict; head odd: via otmp + partition shift DMA
                nc.vector.tensor_mul(opair[pm][0:HD, qsl], otp[0][0:HD, :], invb[0][:])
                otmp = work.tile([HD, QW], bf16, tag="otmp", bufs=2)
                nc.vector.tensor_mul(otmp[:], otp[1][0:HD, :], invb[1][:])
                nc.sync.dma_start(opair[pm][HD:2 * HD, qsl], otmp[:])

        attn_scope.close()
        if phases < 4:
            return

        # ---- output projection: yT[d, :] = sum_m woT[m, d] * opair[m] ----
        out_scope = contextlib.ExitStack()
        psum_out = out_scope.enter_context(
            tc.tile_pool(name="psum_out", bufs=3, space=bass.MemorySpace.PSUM)
        )
        for dcix in range(DC):
            dsl = slice(dcix * PART, (dcix + 1) * PART)
            yts = work.tile([PART, L], bf16, tag="yts", bufs=2)
            for qc in range(4):
                sl = slice(qc * NB, (qc + 1) * NB)
                ps = psum_out.tile([PART, NB], f32, tag="y_ps")
                for mc in range(2):
                    nc.tensor.matmul(
                        ps[:], wo_sb[:, mc, dsl], opair[mc][:, sl],
                        start=(mc == 0), stop=(mc == 1),
                    )
                nc.vector.tensor_copy(yts[:, sl], ps[:])
            nc.sync.dma_start(yT_d.ap()[dsl, :], yts[:])
        out_scope.close()

    import contextlib

    with tile.TileContext(nc) as tc:
        with contextlib.ExitStack() as ctx:
            if n_iter > 1:
                with tc.For_i(
                    0, n_iter, 1,
                    hint_engines=(EngineType.PE, EngineType.Activation,
                                  EngineType.DVE, EngineType.SP),
                ):
                    with contextlib.ExitStack() as ctx2:
                        body(ctx2, tc, phases)
            else:
                body(ctx, tc, phases)

    nc.compile()
    return nc


def prepare_in_maps(inputs):
    """Host-side sharding / folding. Returns per-core input dicts."""
    x = np.asarray(inputs["x"], np.float32)
    gamma = np.asarray(inputs["ln_gamma"], np.float32)
    beta = np.asarray(inputs["ln_beta"], np.float32)
    Wq = np.asarray(inputs["Wq"], np.float32)
    bq = np.asarray(inputs["bq"], np.float32)
    Wk = np.asarray(inputs["Wk"], np.float32)
    bk = np.asarray(inputs["bk"], np.float32)
    Wv = np.asarray(inputs["Wv"], np.float32)
    bv = np.asarray(inputs["bv"], np.float32)
    Wo = np.asarray(inputs["Wo"], np.float32)

    in_maps = []
    for c in range(N_CORES):
        b, g = divmod(c, HG)
        gsl = slice(g * GD, (g + 1) * GD)
        m = {"xT": np.ascontiguousarray(x[b].T).astype(BF16)}
        for name, W, bias in (("q", Wq, bq), ("k", Wk, bk), ("v", Wv, bv)):
            W_eff = (W * gamma[None, :])[gsl]          # [GD, D]
            if name == "v":
                # bv and the beta contribution pass through softmax-normalized
                # attention as a constant row; both fold into bo on the host
                # (see gather_output). Device v needs only the mean term.
                b_eff = np.zeros(GD, np.float32)
            else:
                b_eff = bias[gsl] + W[gsl] @ beta      # [GD]
            wsum = W_eff.sum(axis=1)                   # [GD]
            m[f"w{name}T"] = np.ascontiguousarray(W_eff.T).astype(BF16)
            m[f"corr{name}"] = np.stack([wsum, b_eff]).astype(BF16)
        m["woT"] = np.ascontiguousarray(Wo[:, gsl].T).astype(BF16)
        in_maps.append(m)
    return in_maps


def gather_output(inputs, results):
    x = np.asarray(inputs["x"], np.float32)
    # bv (and beta's contribution through Wv) shift every value row by a
    # constant; softmax rows sum to 1, so the attention output shifts by that
    # same constant and bo absorbs it exactly: bo_eff = bo + Wo @ bv_eff.
    bv_eff = np.asarray(inputs["bv"], np.float32) + (
        np.asarray(inputs["Wv"], np.float32) @ np.asarray(inputs["ln_beta"], np.float32)
    )
    bo = np.asarray(inputs["bo"], np.float32) + (
        np.asarray(inputs["Wo"], np.float32) @ bv_eff
    )
    out = np.empty((B, L, D), np.float32)
    for b in range(B):
        acc = x[b] + bo[None, :]
        for g in range(HG):
            acc = acc + results[b * HG + g]["yT"].astype(np.float32).T
        out[b] = acc
    return out


_PROGRAM_CACHE = {}


def _get_program(n_iter=1, phases=4):
    key = (n_iter, phases)
    if key not in _PROGRAM_CACHE:
        _PROGRAM_CACHE[key] = _build_program(n_iter, phases)
    return _PROGRAM_CACHE[key]


def kernel(**inputs):
    from concourse import bass_utils

    nc = _get_program(1)
    in_maps = prepare_in_maps(inputs)
    res = bass_utils.run_bass_kernel_spmd(nc, in_maps, core_ids=list(range(N_CORES)))
    return gather_output(inputs, res.results)
